# revision 22
# baseline (speedup 1.0000x reference)
"""Trainium2 Bass kernel for an encoder layer (LN -> MHA+bias/mask -> LN -> FFN).

Strategy: pure data parallelism. B=8 batch elements across 8 NeuronCores, one
element per core. The metric is wall-clock per SPMD call over an axon tunnel
(~100 MB/s H2D, ~30 MB/s D2H, ~12 ms per transferred array), so the design
minimizes shipped bytes AND array count per call:

  - ALL per-core inputs travel as ONE fp16 blob [N_ROWS, 512] (~2.8 MB/core),
    byte-punned with AP.bitcast on device:
      rows 0:1024     x fp16 [S, H]
      rows 1024:2048  emb8T: exp'd bias/mask, fp8e4 [S, S] ([k,q] layout)
      rows 2048:2816  weight shard, bf16 (1/8 of wqkvo|w1|w2, AllGathered
                      on-device over NeuronLink into a DRAM bounce buffer)
      rows 2816:2832  folded biases, f32
  - emb = exp(bias - rowmax)*mask*128 is precomputed on host: softmax is
    invariant to per-row scaling, so the rowmax shift + x128 centers the fp8
    dynamic range. Masked entries are exactly 0, so no -1e9 clamp or
    max-subtraction pass is needed on device.
  - v bias folded into the output-projection bias on host (sum p = 1), LN
    affine params folded into W/b as in the reference.
  - output fp16 (halves the donated zero buffer shipped in AND the result
    shipped back), staged in SBUF and written with a single DMA.

Per-core dataflow (S=1024, H=512, NH=8, DH=64, FFN=2048, P=128) is the
transposed-attention scheme: yT built with PE transposes; qT/kT = W.T@yT per
head-pair row-packed (K=64 x2) in the PE array; v_aug carries a ones column
so the PV matmul also produces softmax denominators; FFN keeps hT transposed
so no further transposes are needed. All matmul operands bf16 (full PE rate),
PSUM accumulation fp32.

The gathered weight DRAM layout is rank-interleaved (8 blocks of 768 rows:
wqkvo_s 256 | w1_s 256 | w2_s 256 each); SBUF loads un-interleave it with
strided DMA rearranges. use_collectives=False (CoreSim) ships the full
rank-interleaved weight region in the blob instead, so all downstream code
is identical.
"""

import os
import sys

for _p in ("/opt/trn_rl_repo", "/root/.axon_site/_ro/trn_rl_repo"):
    if os.path.isdir(_p) and _p not in sys.path:
        sys.path.insert(0, _p)

from contextlib import ExitStack

import numpy as np
import ml_dtypes

import concourse.bass as bass
import concourse.tile as tile
from concourse import bacc, mybir
from concourse.masks import make_identity

F32 = mybir.dt.float32
F16 = mybir.dt.float16
BF16 = mybir.dt.bfloat16
F8 = mybir.dt.float8e4
I8 = mybir.dt.int8
AF = mybir.ActivationFunctionType
ALU = mybir.AluOpType

S = 1024
H = 512
NH = 8
DH = 64
FFN = 2048
P = 128
B = 8
EPS = 1e-5
SSC = S // P     # 8 seq tiles of 128
CC = H // P      # 4 channel chunks
FT = FFN // P    # 16 ffn chunks
QC = S // 512    # 2 query chunks of 512

MM_DT = BF16     # matmul-operand dtype (full PE rate, fp32 PSUM accumulate)
EMB_SCALE = 128.0  # per-row softmax scale freedom used to center fp8 range

NP_BF16 = ml_dtypes.bfloat16
NP_F8 = ml_dtypes.float8_e4m3

# blob row offsets (f16 rows of 512 = 1KB each)
R_X = 0
R_EMB = 1024
R_W = 2048
W_SHARD_ROWS = 768          # 256 wqkvo | 256 w1 | 256 w2 (bf16, as f16 rows)
W_FULL_ROWS = B * W_SHARD_ROWS


def _blob_rows(use_collectives):
    wrows = W_SHARD_ROWS if use_collectives else W_FULL_ROWS
    r_ball = R_W + wrows
    r_bbc = r_ball + 12     # ball [128,24] f32 = 12 rows
    n = r_bbc + 4           # bbc [2,512] f32 = 4 rows
    return wrows, r_ball, r_bbc, n


def build_program(use_collectives=True):
    nc = bacc.Bacc(
        "TRN2",
        target_bir_lowering=False,
        debug=False,
        enable_asserts=False,
        num_devices=B,
    )

    wrows, r_ball, r_bbc, n_rows = _blob_rows(use_collectives)
    blob_d = nc.dram_tensor("blob", [n_rows, 512], F16, kind="ExternalInput").ap()
    # out rows 0:1024 int8 delta (out - x, per-seq-row absmax/127 quant),
    # rows 1024:1032 the f32 absmax values (bitcast), laid out [sc*128+p].
    out_d = nc.dram_tensor("out", [S + 8, H], I8, kind="ExternalOutput").ap()

    def _emit(tc, ctx):
        pool = ctx.enter_context(tc.tile_pool(name="main", bufs=1))
        stream = ctx.enter_context(tc.tile_pool(name="stream", bufs=2))
        spool = ctx.enter_context(tc.tile_pool(name="small", bufs=2))
        # PSUM: 2+2+2+2 slots = 8 banks exactly
        ps_mm = ctx.enter_context(tc.tile_pool(name="ps_mm", bufs=2, space="PSUM"))
        ps_s = ctx.enter_context(tc.tile_pool(name="ps_s", bufs=2, space="PSUM"))
        ps_o = ctx.enter_context(tc.tile_pool(name="ps_o", bufs=2, space="PSUM"))
        ps_sm = ctx.enter_context(tc.tile_pool(name="ps_sm", bufs=2, space="PSUM"))

        # ---- gather weight shards into one full rank-interleaved DRAM copy ----
        if use_collectives:
            dpool = ctx.enter_context(tc.tile_pool(name="dram", bufs=1, space="DRAM"))
            bin_t = dpool.tile([W_SHARD_ROWS, 512], MM_DT)
            g_t = dpool.tile([W_FULL_ROWS, 512], MM_DT)
            nc.gpsimd.dma_start(
                bin_t[:], blob_d[R_W:R_W + W_SHARD_ROWS].bitcast(MM_DT)
            )
            nc.gpsimd.collective_compute(
                "AllGather", ALU.bypass, replica_groups=[list(range(B))],
                ins=[bin_t.opt()], outs=[g_t.opt()],
            )
            wg = g_t[:]
        else:
            wg = blob_d[R_W:R_W + W_FULL_ROWS].bitcast(MM_DT)

        # ---- persistent SBUF tensors ----
        ident = pool.tile([P, P], F32, tag="ident")
        make_identity(nc, ident[:])
        x_sb = pool.tile([P, SSC, H], F32, tag="x")        # becomes x2 in place
        delta = pool.tile([P, SSC, H], F32, tag="delta")   # out - x (residual branches)
        scales_sb = pool.tile([P, SSC], F32, tag="scales")
        embT = pool.tile([P, SSC, S], MM_DT, tag="big4mb")  # [k_in, kt, q]
        yT = pool.tile([P, CC, S], MM_DT, tag="yT")          # [c_in, cc, s]
        v_aug = pool.tile([P, SSC, NH, DH + 1], MM_DT, tag="vaug")
        oT = pool.tile([P, CC, S], MM_DT, tag="oT")          # [c_in, cc, s]

        wq_sb = pool.tile([P, CC, H], MM_DT, tag="wslot0")
        wk_sb = pool.tile([P, CC, H], MM_DT, tag="wslot1")
        wv_sb = pool.tile([P, CC, H], MM_DT, tag="wslot2")
        wo_sb = pool.tile([P, CC, H], MM_DT, tag="wslot3")
        b_all = pool.tile([P, 24], F32, tag="ball")        # bq 0:4 | bk 4:8 | b1 8:24
        bo_row = pool.tile([1, H], F32, tag="bo_row")
        b2_row = pool.tile([1, H], F32, tag="b2_row")
        bo_sb = pool.tile([P, H], F32, tag="bo")
        b2_sb = pool.tile([P, H], F32, tag="b2")

        # wqkvo rows g live at rank g//256, inner g%256; [128,512] tiles are
        # 128-row aligned inside 256-row rank chunks: rank 2w+r' holds SBUF
        # chunks cc = 2r', 2r'+1 of weight w (DMA APs max 3 dims).
        for w_i, w_sb in enumerate((wq_sb, wk_sb, wv_sb, wo_sb)):
            for rr in range(2):
                blk = (2 * w_i + rr) * W_SHARD_ROWS
                src = wg[blk:blk + 256]                      # [256, 512]
                src = src.rearrange("(two p) h -> p two h", two=2)
                nc.sync.dma_start(w_sb[:, 2 * rr:2 * rr + 2], src)

        ball_src = blob_d[r_ball:r_ball + 12].bitcast(F32)   # [12, 256]
        ball_src = ball_src.rearrange("a b -> (a b)").rearrange("(p q) -> p q", q=24)
        nc.sync.dma_start(b_all[:], ball_src)
        bbc_src = blob_d[r_bbc:r_bbc + 4].bitcast(F32)       # [4, 256]
        bbc_src = bbc_src.rearrange("a b -> (a b)").rearrange("(p q) -> p q", q=H)
        nc.sync.dma_start(bo_row[:], bbc_src[0:1])
        nc.sync.dma_start(b2_row[:], bbc_src[1:2])
        nc.gpsimd.partition_broadcast(bo_sb[:], bo_row[:])
        nc.gpsimd.partition_broadcast(b2_sb[:], b2_row[:])

        # ---- load x (fp16 -> fp32) and emb (fp8 -> bf16), one DMA each ----
        x16_t = pool.tile([P, SSC, H], F16, tag="x16")
        nc.sync.dma_start(
            x16_t[:],
            blob_d[R_X:R_X + S].rearrange("(sc p) h -> p sc h", p=P),
        )
        nc.vector.tensor_copy(x_sb[:], x16_t[:])
        e8_t = pool.tile([P, SSC, S], F8, tag="e8")
        nc.sync.dma_start(
            e8_t[:],
            blob_d[R_EMB:R_EMB + S].bitcast(F8).rearrange("(kt p) q -> p kt q", p=P),
        )
        nc.scalar.copy(embT[:], e8_t[:])

        # ones columns of v_aug
        ones_col = pool.tile([P, 1], F32, tag="ones_col")
        nc.gpsimd.memset(ones_col[:], 1.0)
        nc.vector.tensor_copy(
            v_aug[:, :, :, DH:DH + 1],
            ones_col[:].to_broadcast((P, SSC, NH, 1)),
        )

        # ---- LN helper: batched stats for all SSC tiles in one pass ----
        def layer_norm_stats(src3):
            """src3: [P, SSC, H] fp32. Returns (rstd, nmr) [P, SSC] tiles with
            y = src*rstd + nmr the per-(partition, sc) normalization."""
            xsq = pool.tile([P, SSC, H], F32, tag="x16")
            nc.vector.tensor_tensor(xsq[:], src3, src3, ALU.mult)
            sumsq = spool.tile([P, SSC], F32, tag="sumsq")
            sumx = spool.tile([P, SSC], F32, tag="sumx")
            nc.vector.reduce_sum(sumsq[:], xsq[:], axis=mybir.AxisListType.X)
            nc.vector.reduce_sum(sumx[:], src3, axis=mybir.AxisListType.X)
            mean = spool.tile([P, SSC], F32, tag="mean")
            nc.vector.tensor_scalar_mul(mean[:], sumx[:], 1.0 / H)
            veps = spool.tile([P, SSC], F32, tag="veps")
            nc.vector.tensor_scalar_mul(veps[:], sumsq[:], 1.0 / H)
            msq = spool.tile([P, SSC], F32, tag="msq")
            nc.vector.tensor_tensor(msq[:], mean[:], mean[:], ALU.mult)
            nc.vector.tensor_tensor(veps[:], veps[:], msq[:], ALU.subtract)
            nc.vector.tensor_scalar_add(veps[:], veps[:], EPS)
            lnv = spool.tile([P, SSC], F32, tag="lnv")
            nc.scalar.activation(lnv[:], veps[:], AF.Ln)
            rstd = spool.tile([P, SSC], F32, tag="rstd")
            # rstd = exp(-0.5*ln(var+eps)); keeps ACT in the exp/ln table set
            nc.scalar.activation(rstd[:], lnv[:], AF.Exp, scale=-0.5)
            nmr = spool.tile([P, SSC], F32, tag="nmr")
            nc.vector.tensor_tensor(nmr[:], mean[:], rstd[:], ALU.mult)
            nc.vector.tensor_scalar_mul(nmr[:], nmr[:], -1.0)
            return rstd, nmr

        def transpose_into(y_tile, dst, sc):
            """PE-transpose y_tile [128, H] into dst [P, CC, S] at seq block sc."""
            for cb in range(CC):
                pst = ps_sm.tile([P, 512], F32, tag="ps_small")
                nc.tensor.transpose(
                    pst[:, 0:P], y_tile[:, cb * P:(cb + 1) * P], ident[:]
                )
                if cb % 2 == 0:
                    nc.scalar.copy(dst[:, cb, sc * P:(sc + 1) * P], pst[:, 0:P])
                else:
                    nc.vector.tensor_copy(dst[:, cb, sc * P:(sc + 1) * P], pst[:, 0:P])

        # ---- LN1 + yT ----
        rstd1, nmr1 = layer_norm_stats(x_sb[:])
        for sc in range(SSC):
            y_t = stream.tile([P, H], F32, tag="y")
            nc.scalar.activation(
                y_t[:], x_sb[:, sc], AF.Identity,
                bias=nmr1[:, sc:sc + 1], scale=rstd1[:, sc:sc + 1],
            )
            transpose_into(y_t, yT, sc)

        # ---- V projection: v[s, h] = yT.T @ Wv, into v_aug slots ----
        for sc in range(SSC):
            psv = ps_mm.tile([P, H], F32, tag="mm")
            for ci in range(CC):
                nc.tensor.matmul(
                    psv[:],
                    yT[:, ci, sc * P:(sc + 1) * P],
                    wv_sb[:, ci],
                    start=(ci == 0),
                    stop=(ci == CC - 1),
                )
            # scatter per head into v_aug[:, sc, h, 0:64] (bv folded into bo')
            nc.vector.tensor_copy(
                v_aug[:, sc, :, 0:DH],
                psv[:].rearrange("p (h d) -> p h d", h=NH),
            )

        # ---- per head-pair: qT/kT projection then attention ----
        with tc.tile_pool(name="attnp", bufs=2) as attnp, \
             tc.tile_pool(name="epool", bufs=4) as epool:
            for cc in range(CC):
                h0, h1 = 2 * cc, 2 * cc + 1
                qT_c = attnp.tile([P, S], MM_DT, tag="qT")
                kT_c = attnp.tile([P, S], MM_DT, tag="kT")
                for qc in range(QC):
                    psq = ps_mm.tile([P, 512], F32, tag="mm")
                    for ci in range(CC):
                        nc.tensor.matmul(
                            psq[:],
                            wq_sb[:, ci, cc * P:(cc + 1) * P],
                            yT[:, ci, qc * 512:(qc + 1) * 512],
                            start=(ci == 0),
                            stop=(ci == CC - 1),
                        )
                    nc.scalar.activation(
                        qT_c[:, qc * 512:(qc + 1) * 512], psq[:], AF.Identity,
                        bias=b_all[:, cc:cc + 1],
                    )
                    psk = ps_mm.tile([P, 512], F32, tag="mm")
                    for ci in range(CC):
                        nc.tensor.matmul(
                            psk[:],
                            wk_sb[:, ci, cc * P:(cc + 1) * P],
                            yT[:, ci, qc * 512:(qc + 1) * 512],
                            start=(ci == 0),
                            stop=(ci == CC - 1),
                        )
                    nc.scalar.activation(
                        kT_c[:, qc * 512:(qc + 1) * 512], psk[:], AF.Identity,
                        bias=b_all[:, 4 + cc:5 + cc],
                    )

                for qc in range(QC):
                    qs = slice(qc * 512, (qc + 1) * 512)
                    # both heads' scores row-packed (K=64 at row groups 0-1 /
                    # 2-3) run concurrently in the PE array; their PV
                    # accumulation groups live in separate PSUM banks.
                    pso0 = ps_o.tile([DH + 1, 512], F32, tag="o", name=f"pso0_{qc}")
                    pso1 = ps_o.tile([DH + 1, 512], F32, tag="o", name=f"pso1_{qc}")
                    for kt in range(SSC):
                        ks = slice(kt * P, (kt + 1) * P)
                        pss0 = ps_s.tile([P, 512], F32, tag="s", name=f"pss0_{qc}_{kt}")
                        pss1 = ps_s.tile([P, 512], F32, tag="s", name=f"pss1_{qc}_{kt}")
                        nc.tensor.matmul(
                            pss0[:], kT_c[0:DH, ks], qT_c[0:DH, qs],
                            tile_position=(0, 0),
                        )
                        nc.tensor.matmul(
                            pss1[:], kT_c[DH:P, ks], qT_c[DH:P, qs],
                            tile_position=(DH, 0),
                        )
                        for h, pss, pso in ((h0, pss0, pso0), (h1, pss1, pso1)):
                            et = epool.tile([P, 512], MM_DT, tag="e", name=f"et_{h}_{qc}_{kt}")
                            nc.scalar.activation(et[:], pss[:], AF.Exp)
                            et2 = epool.tile([P, 512], MM_DT, tag="e2", name=f"et2_{h}_{qc}_{kt}")
                            nc.vector.tensor_tensor(
                                et2[:], et[:], embT[:, kt, qs], ALU.mult
                            )
                            nc.tensor.matmul(
                                pso[:],
                                v_aug[:, kt, h],
                                et2[:],
                                start=(kt == 0),
                                stop=(kt == SSC - 1),
                            )
                    for h, pso in ((h0, pso0), (h1, pso1)):
                        rows = slice(0, DH) if h == h0 else slice(DH, P)
                        dn_t = spool.tile([1, 512], F32, tag="dn")
                        nc.scalar.copy(dn_t[:], pso[DH:DH + 1, :])
                        r_t = spool.tile([1, 512], F32, tag="recip")
                        nc.vector.reciprocal(r_t[:], dn_t[:])
                        bc_t = spool.tile([DH, 512], F32, tag="bc")
                        nc.gpsimd.partition_broadcast(bc_t[:], r_t[:])
                        nc.vector.tensor_tensor(
                            oT[rows, cc, qs], pso[0:DH, :], bc_t[:], ALU.mult
                        )

        # ---- output projection + residual (x2 overwrites x in place) ----
        for sc in range(SSC):
            pso = ps_mm.tile([P, H], F32, tag="mm")
            for ci in range(CC):
                nc.tensor.matmul(
                    pso[:],
                    oT[:, ci, sc * P:(sc + 1) * P],
                    wo_sb[:, ci],
                    start=(ci == 0),
                    stop=(ci == CC - 1),
                )
            nc.vector.tensor_tensor(delta[:, sc], pso[:], bo_sb[:], ALU.add)
            nc.gpsimd.tensor_tensor(x_sb[:, sc], x_sb[:, sc], delta[:, sc], ALU.add)

        # ---- LN2 + y2T (reuses the yT slot) ----
        y2T = pool.tile([P, CC, S], MM_DT, tag="yT")
        rstd2, nmr2 = layer_norm_stats(x_sb[:])
        for sc in range(SSC):
            y_t = stream.tile([P, H], F32, tag="y")
            nc.scalar.activation(
                y_t[:], x_sb[:, sc], AF.Identity,
                bias=nmr2[:, sc:sc + 1], scale=rstd2[:, sc:sc + 1],
            )
            transpose_into(y_t, y2T, sc)

        # ---- FFN ----
        # w1 [512, 2048]: rank r holds w1 rows r*64:(r+1)*64 as 256 blob rows
        # at rank-block offset 256. SBUF [p, cc, f]: p = (r%2)*64 + j, cc = r//2.
        w1_sb = pool.tile([P, CC, FFN], MM_DT, tag="w1")
        for r in range(B):
            src = wg[r * W_SHARD_ROWS + 256:r * W_SHARD_ROWS + 512]
            src = src.rearrange("(j four) h -> j (four h)", four=4)  # [64, 2048]
            nc.sync.dma_start(
                w1_sb[(r % 2) * 64:(r % 2) * 64 + 64, r // 2], src
            )
        # w2 [2048, 512]: rank r holds w2 rows r*256:(r+1)*256 at offset 512.
        w2_sb = pool.tile([P, FT, H], MM_DT, tag="vaug")
        for r in range(B):
            blk = r * W_SHARD_ROWS + 512
            src = wg[blk:blk + 256]                          # [256, 512]
            src = src.rearrange("(two p) h -> p two h", two=2)
            nc.sync.dma_start(w2_sb[:, 2 * r:2 * r + 2], src)
        hT = pool.tile([P, FT, S], MM_DT, tag="big4mb")

        for ft in range(FT):
            for qc in range(QC):
                psh = ps_mm.tile([P, 512], F32, tag="mm")
                for ci in range(CC):
                    nc.tensor.matmul(
                        psh[:],
                        w1_sb[:, ci, ft * P:(ft + 1) * P],
                        y2T[:, ci, qc * 512:(qc + 1) * 512],
                        start=(ci == 0),
                        stop=(ci == CC - 1),
                    )
                nc.scalar.activation(
                    hT[:, ft, qc * 512:(qc + 1) * 512], psh[:], AF.Gelu,
                    bias=b_all[:, 8 + ft:9 + ft],
                )

        o8 = pool.tile([P, SSC, H], I8, tag="x16")
        for sc in range(SSC):
            psf = ps_mm.tile([P, H], F32, tag="mm")
            for ft in range(FT):
                nc.tensor.matmul(
                    psf[:],
                    hT[:, ft, sc * P:(sc + 1) * P],
                    w2_sb[:, ft],
                    start=(ft == 0),
                    stop=(ft == FT - 1),
                )
            # full delta = attn branch + ffn branch; int8 per-row quantize
            d_t = stream.tile([P, H], F32, tag="out_t")
            nc.vector.tensor_tensor(d_t[:], psf[:], delta[:, sc], ALU.add)
            nc.gpsimd.tensor_tensor(d_t[:], d_t[:], b2_sb[:], ALU.add)
            nc.vector.tensor_reduce(
                scales_sb[:, sc:sc + 1], d_t[:], axis=mybir.AxisListType.X,
                op=ALU.max, apply_absolute_value=True,
            )
            r_t = spool.tile([P, 1], F32, tag="qr")
            nc.vector.reciprocal(r_t[:], scales_sb[:, sc:sc + 1])
            nc.vector.tensor_scalar_mul(r_t[:], r_t[:], 127.0)
            nc.scalar.activation(o8[:, sc], d_t[:], AF.Identity, scale=r_t[:])
        nc.sync.dma_start(out_d[0:S].rearrange("(sc p) h -> p sc h", p=P), o8[:])
        sc_dst = out_d[S:S + 8].bitcast(F32)                 # [8, 128]
        sc_dst = sc_dst.rearrange("a b -> (a b)").rearrange("(s p) -> p s", p=P)
        nc.sync.dma_start(sc_dst, scales_sb[:])

    with tile.TileContext(nc) as tc, ExitStack() as ctx:
        _emit(tc, ctx)

    nc.compile()
    return nc


def fold_weights(inputs):
    """Host-side prep: fold LN affine params, attention scale, and the V bias
    into weights/biases. Returns the rank-interleaved weight rows (as f16
    byte view) plus bias rows."""
    f = lambda a: np.asarray(a, np.float32)
    g1, b1l = f(inputs["ln1_g"]), f(inputs["ln1_b"])
    g2, b2l = f(inputs["ln2_g"]), f(inputs["ln2_b"])
    Wq, Wk, Wv, Wo = f(inputs["Wq"]), f(inputs["Wk"]), f(inputs["Wv"]), f(inputs["Wo"])
    W1, W2 = f(inputs["W1"]), f(inputs["W2"])
    scale = DH ** -0.5

    wq = (g1[:, None] * Wq) * scale
    bq = (b1l @ Wq + f(inputs["bq"])) * scale
    wk = g1[:, None] * Wk
    bk = b1l @ Wk + f(inputs["bk"])
    wv = g1[:, None] * Wv
    bv = b1l @ Wv + f(inputs["bv"])
    bo_eff = bv @ Wo + f(inputs["bo"])   # sum_k p_k = 1 folds bv through Wo
    w1 = g2[:, None] * W1
    b1_eff = b2l @ W1 + f(inputs["b1"])
    b2_eff = f(inputs["b2"])

    wqkvo = np.concatenate([wq, wk, wv, Wo], axis=0).astype(NP_BF16)  # [4H, H]
    w1c = w1.astype(NP_BF16)                                          # [H, FFN]
    w2c = W2.astype(NP_BF16)                                          # [FFN, H]
    # rank-interleaved blob rows (f16 byte view, 512 wide):
    # rank r block = [wqkvo rows r*256 | w1 rows r*64 (as 256 rows) | w2 rows r*256]
    shards = []
    for r in range(B):
        shards.append(np.concatenate([
            wqkvo[r * 256:(r + 1) * 256].view(np.float16),
            w1c[r * 64:(r + 1) * 64].view(np.float16).reshape(256, 512),
            w2c[r * 256:(r + 1) * 256].view(np.float16),
        ], axis=0))
    ball = np.zeros((P, 24), np.float32)
    ball[:, 0:4] = bq.reshape(CC, P).T
    ball[:, 4:8] = bk.reshape(CC, P).T
    ball[:, 8:24] = b1_eff.reshape(FT, P).T
    bias_rows = np.concatenate([
        ball.view(np.float16).reshape(12, 512),
        np.stack([bo_eff, b2_eff]).astype(np.float32).view(np.float16).reshape(4, 512),
    ], axis=0)
    return shards, bias_rows


def make_in_maps(inputs, use_collectives=True):
    """Build per-core input dicts: one fp16 blob per core."""
    shards, bias_rows = fold_weights(inputs)
    x = np.asarray(inputs["x"], np.float32)
    ab = np.asarray(inputs["attn_bias"], np.float32)
    gm = np.asarray(inputs["graph_mask"]) != 0            # [B, q, k]

    # emb[q,k] = exp(ab - rowmax_q) * mask * EMB_SCALE, in (0, EMB_SCALE].
    # Softmax over k is invariant to any per-q scale, so this only centers
    # the fp8 dynamic range (avoids both overflow and subnormal crush).
    masked = np.where(gm, ab, -np.inf)
    rowmax = masked.max(axis=-1, keepdims=True)           # [B, q, 1]
    rowmax = np.where(np.isfinite(rowmax), rowmax, 0.0)
    emb = np.exp(ab - rowmax) * gm * EMB_SCALE
    embT = np.swapaxes(emb, 1, 2)                         # [B, k, q]
    emb8 = np.ascontiguousarray(embT).astype(NP_F8)

    wrows, r_ball, r_bbc, n_rows = _blob_rows(use_collectives)
    in_maps = []
    for b in range(B):
        blob = np.empty((n_rows, 512), np.float16)
        blob[R_X:R_X + S] = x[b].astype(np.float16)
        blob[R_EMB:R_EMB + S] = emb8[b].view(np.float16)
        if use_collectives:
            blob[R_W:R_W + W_SHARD_ROWS] = shards[b]
        else:
            blob[R_W:R_W + W_FULL_ROWS] = np.concatenate(shards, axis=0)
        blob[r_ball:n_rows] = bias_rows
        in_maps.append({"blob": blob})
    return in_maps


_NC_CACHE = {}


def _get_nc(use_collectives=True):
    key = ("nc", use_collectives)
    if key not in _NC_CACHE:
        _NC_CACHE[key] = build_program(use_collectives)
    return _NC_CACHE[key]


def decode_out(raw, x_b):
    """raw: [S+8, 512] int8 device output -> fp32 [S, H] result.
    rows 0:S = int8 delta rows (seq-ordered), rows S:S+8 = f32 absmax
    per seq row (bitcast, seq-ordered). out = x + delta*absmax/127."""
    raw = np.asarray(raw)
    q = raw[0:S].astype(np.float32)
    absmax = np.frombuffer(raw[S:S + 8].tobytes(), np.float32)  # [S]
    return np.asarray(x_b, np.float32) + q * (absmax / 127.0)[:, None]


def kernel(**inputs) -> np.ndarray:
    from concourse import bass_utils

    nc = _get_nc()
    in_maps = make_in_maps(inputs)
    res = bass_utils.run_bass_kernel_spmd(nc, in_maps, core_ids=list(range(B)))
    x = np.asarray(inputs["x"], np.float32)
    return np.stack(
        [decode_out(res.results[b]["out"], x[b]) for b in range(B)], axis=0
    )


if __name__ == "__main__":
    nc = build_program()
    print("build+compile OK")


# revision 28
# speedup vs baseline: 1.0495x; 1.0495x over previous
"""Trainium2 Bass kernel for an encoder layer (LN -> MHA+bias/mask -> LN -> FFN).

Strategy: pure data parallelism. B=8 batch elements across 8 NeuronCores, one
element per core. The metric is wall-clock per SPMD call over an axon tunnel
(~100 MB/s H2D, ~30 MB/s D2H, ~12 ms per transferred array), so the design
minimizes shipped bytes AND array count per call:

  - ALL per-core inputs travel as ONE fp16 blob [N_ROWS, 512] (~2.8 MB/core),
    byte-punned with AP.bitcast on device:
      rows 0:1024     x fp16 [S, H]
      rows 1024:2048  emb8T: exp'd bias/mask, fp8e4 [S, S] ([k,q] layout)
      rows 2048:2816  weight shard, bf16 (1/8 of wqkvo|w1|w2, AllGathered
                      on-device over NeuronLink into a DRAM bounce buffer)
      rows 2816:2832  folded biases, f32
  - emb = exp(bias - rowmax)*mask*128 is precomputed on host: softmax is
    invariant to per-row scaling, so the rowmax shift + x128 centers the fp8
    dynamic range. Masked entries are exactly 0, so no -1e9 clamp or
    max-subtraction pass is needed on device.
  - v bias folded into the output-projection bias on host (sum p = 1), LN
    affine params folded into W/b as in the reference.
  - output fp16 (halves the donated zero buffer shipped in AND the result
    shipped back), staged in SBUF and written with a single DMA.

Per-core dataflow (S=1024, H=512, NH=8, DH=64, FFN=2048, P=128) is the
transposed-attention scheme: yT built with PE transposes; qT/kT = W.T@yT per
head-pair row-packed (K=64 x2) in the PE array; v_aug carries a ones column
so the PV matmul also produces softmax denominators; FFN keeps hT transposed
so no further transposes are needed. All matmul operands bf16 (full PE rate),
PSUM accumulation fp32.

The gathered weight DRAM layout is rank-interleaved (8 blocks of 768 rows:
wqkvo_s 256 | w1_s 256 | w2_s 256 each); SBUF loads un-interleave it with
strided DMA rearranges. use_collectives=False (CoreSim) ships the full
rank-interleaved weight region in the blob instead, so all downstream code
is identical.
"""

import os
import sys

for _p in ("/opt/trn_rl_repo", "/root/.axon_site/_ro/trn_rl_repo"):
    if os.path.isdir(_p) and _p not in sys.path:
        sys.path.insert(0, _p)

from contextlib import ExitStack

import numpy as np
import ml_dtypes

import concourse.bass as bass
import concourse.tile as tile
from concourse import bacc, mybir
from concourse.masks import make_identity

F32 = mybir.dt.float32
F16 = mybir.dt.float16
BF16 = mybir.dt.bfloat16
F8 = mybir.dt.float8e4
I8 = mybir.dt.int8
AF = mybir.ActivationFunctionType
ALU = mybir.AluOpType

S = 1024
H = 512
NH = 8
DH = 64
FFN = 2048
P = 128
B = 8
EPS = 1e-5
SSC = S // P     # 8 seq tiles of 128
CC = H // P      # 4 channel chunks
FT = FFN // P    # 16 ffn chunks
QC = S // 512    # 2 query chunks of 512

MM_DT = BF16     # matmul-operand dtype (full PE rate, fp32 PSUM accumulate)
EMB_SCALE = 128.0  # per-row softmax scale freedom used to center fp8 range

NP_BF16 = ml_dtypes.bfloat16
NP_F8 = ml_dtypes.float8_e4m3

# blob row offsets (f16 rows of 512 = 1KB each)
R_X = 0                     # x int8 [S, H] = 512 rows (bitcast)
R_EMB = 512
R_W = 1536
W_SHARD_ROWS = 768          # 256 wqkvo | 256 w1 | 256 w2 (bf16, as f16 rows)
W_FULL_ROWS = B * W_SHARD_ROWS


def _blob_rows(use_collectives):
    wrows = W_SHARD_ROWS if use_collectives else W_FULL_ROWS
    r_xs = R_W + wrows      # x per-row absmax f32 [S] = 4 rows
    r_ball = r_xs + 4
    r_bbc = r_ball + 12     # ball [128,24] f32 = 12 rows
    n = r_bbc + 4           # bbc [2,512] f32 = 4 rows
    return wrows, r_xs, r_ball, r_bbc, n


def build_program(use_collectives=True):
    nc = bacc.Bacc(
        "TRN2",
        target_bir_lowering=False,
        debug=False,
        enable_asserts=False,
        num_devices=B,
    )

    wrows, r_xs, r_ball, r_bbc, n_rows = _blob_rows(use_collectives)
    blob_d = nc.dram_tensor("blob", [n_rows, 512], F16, kind="ExternalInput").ap()
    # out rows 0:1024 int8 delta (out - x, per-seq-row absmax/127 quant),
    # rows 1024:1032 the f32 absmax values (bitcast), laid out [sc*128+p].
    out_d = nc.dram_tensor("out", [S + 8, H], I8, kind="ExternalOutput").ap()

    def _emit(tc, ctx):
        pool = ctx.enter_context(tc.tile_pool(name="main", bufs=1))
        stream = ctx.enter_context(tc.tile_pool(name="stream", bufs=2))
        spool = ctx.enter_context(tc.tile_pool(name="small", bufs=2))
        # PSUM: 2+2+2+2 slots = 8 banks exactly
        ps_mm = ctx.enter_context(tc.tile_pool(name="ps_mm", bufs=2, space="PSUM"))
        ps_s = ctx.enter_context(tc.tile_pool(name="ps_s", bufs=2, space="PSUM"))
        ps_o = ctx.enter_context(tc.tile_pool(name="ps_o", bufs=2, space="PSUM"))
        ps_sm = ctx.enter_context(tc.tile_pool(name="ps_sm", bufs=2, space="PSUM"))

        # ---- gather weight shards into one full rank-interleaved DRAM copy ----
        if use_collectives:
            dpool = ctx.enter_context(tc.tile_pool(name="dram", bufs=1, space="DRAM"))
            bin_t = dpool.tile([W_SHARD_ROWS, 512], MM_DT)
            g_t = dpool.tile([W_FULL_ROWS, 512], MM_DT)
            nc.gpsimd.dma_start(
                bin_t[:], blob_d[R_W:R_W + W_SHARD_ROWS].bitcast(MM_DT)
            )
            nc.gpsimd.collective_compute(
                "AllGather", ALU.bypass, replica_groups=[list(range(B))],
                ins=[bin_t.opt()], outs=[g_t.opt()],
            )
            wg = g_t[:]
        else:
            wg = blob_d[R_W:R_W + W_FULL_ROWS].bitcast(MM_DT)

        # ---- persistent SBUF tensors ----
        ident = pool.tile([P, P], F32, tag="ident")
        make_identity(nc, ident[:])
        x_sb = pool.tile([P, SSC, H], F32, tag="x")        # becomes x2 in place
        delta = pool.tile([P, SSC, H], F32, tag="delta")   # out - x (residual branches)
        scales_sb = pool.tile([P, SSC], F32, tag="scales")
        embT = pool.tile([P, SSC, S], MM_DT, tag="big4mb")  # [k_in, kt, q]
        yT = pool.tile([P, CC, S], MM_DT, tag="yT")          # [c_in, cc, s]
        v_aug = pool.tile([P, SSC, NH, DH + 1], MM_DT, tag="vaug")
        oT = pool.tile([P, CC, S], MM_DT, tag="oT")          # [c_in, cc, s]

        wq_sb = pool.tile([P, CC, H], MM_DT, tag="wslot0")
        wk_sb = pool.tile([P, CC, H], MM_DT, tag="wslot1")
        wv_sb = pool.tile([P, CC, H], MM_DT, tag="wslot2")
        wo_sb = pool.tile([P, CC, H], MM_DT, tag="wslot3")
        b_all = pool.tile([P, 24], F32, tag="ball")        # bq 0:4 | bk 4:8 | b1 8:24
        bo_row = pool.tile([1, H], F32, tag="bo_row")
        b2_row = pool.tile([1, H], F32, tag="b2_row")
        bo_sb = pool.tile([P, H], F32, tag="bo")
        b2_sb = pool.tile([P, H], F32, tag="b2")

        # wqkvo rows g live at rank g//256, inner g%256; [128,512] tiles are
        # 128-row aligned inside 256-row rank chunks: rank 2w+r' holds SBUF
        # chunks cc = 2r', 2r'+1 of weight w (DMA APs max 3 dims).
        for w_i, w_sb in enumerate((wq_sb, wk_sb, wv_sb, wo_sb)):
            for rr in range(2):
                blk = (2 * w_i + rr) * W_SHARD_ROWS
                src = wg[blk:blk + 256]                      # [256, 512]
                src = src.rearrange("(two p) h -> p two h", two=2)
                nc.sync.dma_start(w_sb[:, 2 * rr:2 * rr + 2], src)

        ball_src = blob_d[r_ball:r_ball + 12].bitcast(F32)   # [12, 256]
        ball_src = ball_src.rearrange("a b -> (a b)").rearrange("(p q) -> p q", q=24)
        nc.sync.dma_start(b_all[:], ball_src)
        bbc_src = blob_d[r_bbc:r_bbc + 4].bitcast(F32)       # [4, 256]
        bbc_src = bbc_src.rearrange("a b -> (a b)").rearrange("(p q) -> p q", q=H)
        nc.sync.dma_start(bo_row[:], bbc_src[0:1])
        nc.sync.dma_start(b2_row[:], bbc_src[1:2])
        nc.gpsimd.partition_broadcast(bo_sb[:], bo_row[:])
        nc.gpsimd.partition_broadcast(b2_sb[:], b2_row[:])

        # ---- load x (int8 + per-row absmax -> fp32) and emb (fp8 -> bf16) ----
        xq_t = pool.tile([P, SSC, H], I8, tag="x16")
        xq_src = blob_d[R_X:R_X + S // 2].bitcast(I8)        # [512, 1024]
        xq_src = xq_src.rearrange("a (two h) -> (a two) h", two=2)  # x rows [1024, 512]
        nc.sync.dma_start(
            xq_t[:], xq_src.rearrange("(sc p) h -> p sc h", p=P)
        )
        xsc_sb = pool.tile([P, SSC], F32, tag="xsc")
        xs_src = blob_d[r_xs:r_xs + 4].bitcast(F32)          # [4, 128]
        xs_src = xs_src.rearrange("a b -> (a b)").rearrange("(sc p) -> p sc", p=P)
        nc.sync.dma_start(xsc_sb[:], xs_src)
        nc.vector.tensor_scalar_mul(xsc_sb[:], xsc_sb[:], 1.0 / 127.0)
        for sc in range(SSC):
            nc.scalar.activation(
                x_sb[:, sc], xq_t[:, sc], AF.Identity, scale=xsc_sb[:, sc:sc + 1]
            )
        e8_t = pool.tile([P, SSC, S], F8, tag="e8")
        nc.sync.dma_start(
            e8_t[:],
            blob_d[R_EMB:R_EMB + S].bitcast(F8).rearrange("(kt p) q -> p kt q", p=P),
        )
        nc.scalar.copy(embT[:], e8_t[:])

        # ones columns of v_aug
        ones_col = pool.tile([P, 1], F32, tag="ones_col")
        nc.gpsimd.memset(ones_col[:], 1.0)
        nc.vector.tensor_copy(
            v_aug[:, :, :, DH:DH + 1],
            ones_col[:].to_broadcast((P, SSC, NH, 1)),
        )

        # ---- LN helper: batched stats for all SSC tiles in one pass ----
        def layer_norm_stats(src3):
            """src3: [P, SSC, H] fp32. Returns (rstd, nmr) [P, SSC] tiles with
            y = src*rstd + nmr the per-(partition, sc) normalization."""
            xsq = pool.tile([P, SSC, H], F32, tag="x16")
            nc.vector.tensor_tensor(xsq[:], src3, src3, ALU.mult)
            sumsq = spool.tile([P, SSC], F32, tag="sumsq")
            sumx = spool.tile([P, SSC], F32, tag="sumx")
            nc.vector.reduce_sum(sumsq[:], xsq[:], axis=mybir.AxisListType.X)
            nc.vector.reduce_sum(sumx[:], src3, axis=mybir.AxisListType.X)
            mean = spool.tile([P, SSC], F32, tag="mean")
            nc.vector.tensor_scalar_mul(mean[:], sumx[:], 1.0 / H)
            veps = spool.tile([P, SSC], F32, tag="veps")
            nc.vector.tensor_scalar_mul(veps[:], sumsq[:], 1.0 / H)
            msq = spool.tile([P, SSC], F32, tag="msq")
            nc.vector.tensor_tensor(msq[:], mean[:], mean[:], ALU.mult)
            nc.vector.tensor_tensor(veps[:], veps[:], msq[:], ALU.subtract)
            nc.vector.tensor_scalar_add(veps[:], veps[:], EPS)
            lnv = spool.tile([P, SSC], F32, tag="lnv")
            nc.scalar.activation(lnv[:], veps[:], AF.Ln)
            rstd = spool.tile([P, SSC], F32, tag="rstd")
            # rstd = exp(-0.5*ln(var+eps)); keeps ACT in the exp/ln table set
            nc.scalar.activation(rstd[:], lnv[:], AF.Exp, scale=-0.5)
            nmr = spool.tile([P, SSC], F32, tag="nmr")
            nc.vector.tensor_tensor(nmr[:], mean[:], rstd[:], ALU.mult)
            nc.vector.tensor_scalar_mul(nmr[:], nmr[:], -1.0)
            return rstd, nmr

        def transpose_into(y_tile, dst, sc):
            """PE-transpose y_tile [128, H] into dst [P, CC, S] at seq block sc.
            All CC transposes land in disjoint columns of one PSUM tile so a
            single strided copy evacuates them."""
            pst = ps_sm.tile([P, 512], F32, tag="ps_small")
            for cb in range(CC):
                nc.tensor.transpose(
                    pst[:, cb * P:(cb + 1) * P], y_tile[:, cb * P:(cb + 1) * P],
                    ident[:],
                )
            src = pst[:].rearrange("p (cb c) -> p cb c", cb=CC)
            if sc % 2 == 0:
                nc.scalar.copy(dst[:, :, sc * P:(sc + 1) * P], src)
            else:
                nc.vector.tensor_copy(dst[:, :, sc * P:(sc + 1) * P], src)

        # ---- LN1 + yT ----
        rstd1, nmr1 = layer_norm_stats(x_sb[:])
        for sc in range(SSC):
            y_t = stream.tile([P, H], F32, tag="y")
            nc.scalar.activation(
                y_t[:], x_sb[:, sc], AF.Identity,
                bias=nmr1[:, sc:sc + 1], scale=rstd1[:, sc:sc + 1],
            )
            transpose_into(y_t, yT, sc)

        # ---- V projection: v[s, h] = yT.T @ Wv, into v_aug slots ----
        for sc in range(SSC):
            psv = ps_mm.tile([P, H], F32, tag="mm")
            for ci in range(CC):
                nc.tensor.matmul(
                    psv[:],
                    yT[:, ci, sc * P:(sc + 1) * P],
                    wv_sb[:, ci],
                    start=(ci == 0),
                    stop=(ci == CC - 1),
                )
            # scatter per head into v_aug[:, sc, h, 0:64] (bv folded into bo')
            nc.vector.tensor_copy(
                v_aug[:, sc, :, 0:DH],
                psv[:].rearrange("p (h d) -> p h d", h=NH),
            )

        # ---- per head-pair: qT/kT projection then attention ----
        with tc.tile_pool(name="attnp", bufs=2) as attnp, \
             tc.tile_pool(name="epool", bufs=4) as epool:
            for cc in range(CC):
                h0, h1 = 2 * cc, 2 * cc + 1
                qT_c = attnp.tile([P, S], MM_DT, tag="qT")
                kT_c = attnp.tile([P, S], MM_DT, tag="kT")
                for qc in range(QC):
                    psq = ps_mm.tile([P, 512], F32, tag="mm")
                    for ci in range(CC):
                        nc.tensor.matmul(
                            psq[:],
                            wq_sb[:, ci, cc * P:(cc + 1) * P],
                            yT[:, ci, qc * 512:(qc + 1) * 512],
                            start=(ci == 0),
                            stop=(ci == CC - 1),
                        )
                    nc.scalar.activation(
                        qT_c[:, qc * 512:(qc + 1) * 512], psq[:], AF.Identity,
                        bias=b_all[:, cc:cc + 1],
                    )
                    psk = ps_mm.tile([P, 512], F32, tag="mm")
                    for ci in range(CC):
                        nc.tensor.matmul(
                            psk[:],
                            wk_sb[:, ci, cc * P:(cc + 1) * P],
                            yT[:, ci, qc * 512:(qc + 1) * 512],
                            start=(ci == 0),
                            stop=(ci == CC - 1),
                        )
                    nc.scalar.activation(
                        kT_c[:, qc * 512:(qc + 1) * 512], psk[:], AF.Identity,
                        bias=b_all[:, 4 + cc:5 + cc],
                    )

                for qc in range(QC):
                    qs = slice(qc * 512, (qc + 1) * 512)
                    # both heads' scores row-packed (K=64 at row groups 0-1 /
                    # 2-3) run concurrently in the PE array; their PV
                    # accumulation groups live in separate PSUM banks.
                    pso0 = ps_o.tile([DH + 1, 512], F32, tag="o", name=f"pso0_{qc}")
                    pso1 = ps_o.tile([DH + 1, 512], F32, tag="o", name=f"pso1_{qc}")
                    for kt in range(SSC):
                        ks = slice(kt * P, (kt + 1) * P)
                        pss0 = ps_s.tile([P, 512], F32, tag="s", name=f"pss0_{qc}_{kt}")
                        pss1 = ps_s.tile([P, 512], F32, tag="s", name=f"pss1_{qc}_{kt}")
                        nc.tensor.matmul(
                            pss0[:], kT_c[0:DH, ks], qT_c[0:DH, qs],
                            tile_position=(0, 0),
                        )
                        nc.tensor.matmul(
                            pss1[:], kT_c[DH:P, ks], qT_c[DH:P, qs],
                            tile_position=(DH, 0),
                        )
                        for h, pss, pso in ((h0, pss0, pso0), (h1, pss1, pso1)):
                            et = epool.tile([P, 512], MM_DT, tag="e", name=f"et_{h}_{qc}_{kt}")
                            nc.scalar.activation(et[:], pss[:], AF.Exp)
                            et2 = epool.tile([P, 512], MM_DT, tag="e2", name=f"et2_{h}_{qc}_{kt}")
                            nc.vector.tensor_tensor(
                                et2[:], et[:], embT[:, kt, qs], ALU.mult
                            )
                            nc.tensor.matmul(
                                pso[:],
                                v_aug[:, kt, h],
                                et2[:],
                                start=(kt == 0),
                                stop=(kt == SSC - 1),
                            )
                    for h, pso in ((h0, pso0), (h1, pso1)):
                        rows = slice(0, DH) if h == h0 else slice(DH, P)
                        dn_t = spool.tile([1, 512], F32, tag="dn")
                        nc.scalar.copy(dn_t[:], pso[DH:DH + 1, :])
                        r_t = spool.tile([1, 512], F32, tag="recip")
                        nc.vector.reciprocal(r_t[:], dn_t[:])
                        bc_t = spool.tile([DH, 512], F32, tag="bc")
                        nc.gpsimd.partition_broadcast(bc_t[:], r_t[:])
                        nc.vector.tensor_tensor(
                            oT[rows, cc, qs], pso[0:DH, :], bc_t[:], ALU.mult
                        )

        # ---- output projection + residual (x2 overwrites x in place) ----
        for sc in range(SSC):
            pso = ps_mm.tile([P, H], F32, tag="mm")
            for ci in range(CC):
                nc.tensor.matmul(
                    pso[:],
                    oT[:, ci, sc * P:(sc + 1) * P],
                    wo_sb[:, ci],
                    start=(ci == 0),
                    stop=(ci == CC - 1),
                )
            nc.vector.tensor_tensor(delta[:, sc], pso[:], bo_sb[:], ALU.add)
            nc.gpsimd.tensor_tensor(x_sb[:, sc], x_sb[:, sc], delta[:, sc], ALU.add)

        # ---- LN2 + y2T (reuses the yT slot) ----
        y2T = pool.tile([P, CC, S], MM_DT, tag="yT")
        rstd2, nmr2 = layer_norm_stats(x_sb[:])
        for sc in range(SSC):
            y_t = stream.tile([P, H], F32, tag="y")
            nc.scalar.activation(
                y_t[:], x_sb[:, sc], AF.Identity,
                bias=nmr2[:, sc:sc + 1], scale=rstd2[:, sc:sc + 1],
            )
            transpose_into(y_t, y2T, sc)

        # ---- FFN ----
        # w1 [512, 2048]: rank r holds w1 rows r*64:(r+1)*64 as 256 blob rows
        # at rank-block offset 256. SBUF [p, cc, f]: p = (r%2)*64 + j, cc = r//2.
        w1_sb = pool.tile([P, CC, FFN], MM_DT, tag="w1")
        for r in range(B):
            src = wg[r * W_SHARD_ROWS + 256:r * W_SHARD_ROWS + 512]
            src = src.rearrange("(j four) h -> j (four h)", four=4)  # [64, 2048]
            nc.sync.dma_start(
                w1_sb[(r % 2) * 64:(r % 2) * 64 + 64, r // 2], src
            )
        # w2 [2048, 512]: rank r holds w2 rows r*256:(r+1)*256 at offset 512.
        w2_sb = pool.tile([P, FT, H], MM_DT, tag="vaug")
        for r in range(B):
            blk = r * W_SHARD_ROWS + 512
            src = wg[blk:blk + 256]                          # [256, 512]
            src = src.rearrange("(two p) h -> p two h", two=2)
            nc.sync.dma_start(w2_sb[:, 2 * r:2 * r + 2], src)
        hT = pool.tile([P, FT, S], MM_DT, tag="big4mb")

        for ft in range(FT):
            for qc in range(QC):
                psh = ps_mm.tile([P, 512], F32, tag="mm")
                for ci in range(CC):
                    nc.tensor.matmul(
                        psh[:],
                        w1_sb[:, ci, ft * P:(ft + 1) * P],
                        y2T[:, ci, qc * 512:(qc + 1) * 512],
                        start=(ci == 0),
                        stop=(ci == CC - 1),
                    )
                nc.scalar.activation(
                    hT[:, ft, qc * 512:(qc + 1) * 512], psh[:], AF.Gelu,
                    bias=b_all[:, 8 + ft:9 + ft],
                )

        o8 = pool.tile([P, SSC, H], I8, tag="x16")
        for sc in range(SSC):
            psf = ps_mm.tile([P, H], F32, tag="mm")
            for ft in range(FT):
                nc.tensor.matmul(
                    psf[:],
                    hT[:, ft, sc * P:(sc + 1) * P],
                    w2_sb[:, ft],
                    start=(ft == 0),
                    stop=(ft == FT - 1),
                )
            # full delta = attn branch + ffn branch; int8 per-row quantize
            d_t = stream.tile([P, H], F32, tag="out_t")
            nc.vector.tensor_tensor(d_t[:], psf[:], delta[:, sc], ALU.add)
            nc.gpsimd.tensor_tensor(d_t[:], d_t[:], b2_sb[:], ALU.add)
            nc.vector.tensor_reduce(
                scales_sb[:, sc:sc + 1], d_t[:], axis=mybir.AxisListType.X,
                op=ALU.max, apply_absolute_value=True,
            )
            r_t = spool.tile([P, 1], F32, tag="qr")
            nc.vector.reciprocal(r_t[:], scales_sb[:, sc:sc + 1])
            nc.vector.tensor_scalar_mul(r_t[:], r_t[:], 127.0)
            nc.scalar.activation(o8[:, sc], d_t[:], AF.Identity, scale=r_t[:])
        nc.sync.dma_start(out_d[0:S].rearrange("(sc p) h -> p sc h", p=P), o8[:])
        sc_dst = out_d[S:S + 8].bitcast(F32)                 # [8, 128]
        sc_dst = sc_dst.rearrange("a b -> (a b)").rearrange("(s p) -> p s", p=P)
        nc.sync.dma_start(sc_dst, scales_sb[:])

    with tile.TileContext(nc) as tc, ExitStack() as ctx:
        _emit(tc, ctx)

    nc.compile()
    return nc


def fold_weights(inputs):
    """Host-side prep: fold LN affine params, attention scale, and the V bias
    into weights/biases. Returns the rank-interleaved weight rows (as f16
    byte view) plus bias rows."""
    f = lambda a: np.asarray(a, np.float32)
    g1, b1l = f(inputs["ln1_g"]), f(inputs["ln1_b"])
    g2, b2l = f(inputs["ln2_g"]), f(inputs["ln2_b"])
    Wq, Wk, Wv, Wo = f(inputs["Wq"]), f(inputs["Wk"]), f(inputs["Wv"]), f(inputs["Wo"])
    W1, W2 = f(inputs["W1"]), f(inputs["W2"])
    scale = DH ** -0.5

    wq = (g1[:, None] * Wq) * scale
    bq = (b1l @ Wq + f(inputs["bq"])) * scale
    wk = g1[:, None] * Wk
    bk = b1l @ Wk + f(inputs["bk"])
    wv = g1[:, None] * Wv
    bv = b1l @ Wv + f(inputs["bv"])
    bo_eff = bv @ Wo + f(inputs["bo"])   # sum_k p_k = 1 folds bv through Wo
    w1 = g2[:, None] * W1
    b1_eff = b2l @ W1 + f(inputs["b1"])
    b2_eff = f(inputs["b2"])

    wqkvo = np.concatenate([wq, wk, wv, Wo], axis=0).astype(NP_BF16)  # [4H, H]
    w1c = w1.astype(NP_BF16)                                          # [H, FFN]
    w2c = W2.astype(NP_BF16)                                          # [FFN, H]
    # rank-interleaved blob rows (f16 byte view, 512 wide):
    # rank r block = [wqkvo rows r*256 | w1 rows r*64 (as 256 rows) | w2 rows r*256]
    shards = []
    for r in range(B):
        shards.append(np.concatenate([
            wqkvo[r * 256:(r + 1) * 256].view(np.float16),
            w1c[r * 64:(r + 1) * 64].view(np.float16).reshape(256, 512),
            w2c[r * 256:(r + 1) * 256].view(np.float16),
        ], axis=0))
    ball = np.zeros((P, 24), np.float32)
    ball[:, 0:4] = bq.reshape(CC, P).T
    ball[:, 4:8] = bk.reshape(CC, P).T
    ball[:, 8:24] = b1_eff.reshape(FT, P).T
    bias_rows = np.concatenate([
        ball.view(np.float16).reshape(12, 512),
        np.stack([bo_eff, b2_eff]).astype(np.float32).view(np.float16).reshape(4, 512),
    ], axis=0)
    return shards, bias_rows


def make_in_maps(inputs, use_collectives=True):
    """Build per-core input dicts: one fp16 blob per core."""
    shards, bias_rows = fold_weights(inputs)
    x = np.asarray(inputs["x"], np.float32)
    ab = np.asarray(inputs["attn_bias"], np.float32)
    gm = np.asarray(inputs["graph_mask"]) != 0            # [B, q, k]

    # emb[q,k] = exp(ab - rowmax_q) * mask * EMB_SCALE, in (0, EMB_SCALE].
    # Softmax over k is invariant to any per-q scale, so this only centers
    # the fp8 dynamic range (avoids both overflow and subnormal crush).
    masked = np.where(gm, ab, -np.inf)
    rowmax = masked.max(axis=-1, keepdims=True)           # [B, q, 1]
    rowmax = np.where(np.isfinite(rowmax), rowmax, 0.0)
    emb = np.exp(ab - rowmax) * gm * EMB_SCALE
    embT = np.swapaxes(emb, 1, 2)                         # [B, k, q]
    emb8 = np.ascontiguousarray(embT).astype(NP_F8)

    # x: per-seq-row absmax int8 quantization (reconstructed on device)
    x_absmax = np.abs(x).max(axis=-1)                     # [B, S]
    x_absmax = np.maximum(x_absmax, 1e-9)
    xq = np.clip(
        np.rint(x * (127.0 / x_absmax[..., None])), -127, 127
    ).astype(np.int8)                                     # [B, S, H]

    wrows, r_xs, r_ball, r_bbc, n_rows = _blob_rows(use_collectives)
    in_maps = []
    for b in range(B):
        blob = np.empty((n_rows, 512), np.float16)
        blob[R_X:R_X + S // 2] = xq[b].reshape(S // 2, 1024).view(np.float16)
        blob[R_EMB:R_EMB + S] = emb8[b].view(np.float16)
        if use_collectives:
            blob[R_W:R_W + W_SHARD_ROWS] = shards[b]
        else:
            blob[R_W:R_W + W_FULL_ROWS] = np.concatenate(shards, axis=0)
        blob[r_xs:r_xs + 4] = (
            x_absmax[b].astype(np.float32).view(np.float16).reshape(4, 512)
        )
        blob[r_ball:n_rows] = bias_rows
        in_maps.append({"blob": blob})
    return in_maps


_NC_CACHE = {}


def _get_nc(use_collectives=True):
    key = ("nc", use_collectives)
    if key not in _NC_CACHE:
        _NC_CACHE[key] = build_program(use_collectives)
    return _NC_CACHE[key]


def decode_out(raw, x_b):
    """raw: [S+8, 512] int8 device output -> fp32 [S, H] result.
    rows 0:S = int8 delta rows (seq-ordered), rows S:S+8 = f32 absmax
    per seq row (bitcast, seq-ordered). out = x + delta*absmax/127."""
    raw = np.asarray(raw)
    q = raw[0:S].astype(np.float32)
    absmax = np.frombuffer(raw[S:S + 8].tobytes(), np.float32)  # [S]
    return np.asarray(x_b, np.float32) + q * (absmax / 127.0)[:, None]


def kernel(**inputs) -> np.ndarray:
    from concourse import bass_utils

    nc = _get_nc()
    in_maps = make_in_maps(inputs)
    res = bass_utils.run_bass_kernel_spmd(nc, in_maps, core_ids=list(range(B)))
    x = np.asarray(inputs["x"], np.float32)
    return np.stack(
        [decode_out(res.results[b]["out"], x[b]) for b in range(B)], axis=0
    )


if __name__ == "__main__":
    nc = build_program()
    print("build+compile OK")


# revision 31
# speedup vs baseline: 1.1049x; 1.0528x over previous
"""Trainium2 Bass kernel for an encoder layer (LN -> MHA+bias/mask -> LN -> FFN).

Strategy: pure data parallelism. B=8 batch elements across 8 NeuronCores, one
element per core. The metric is wall-clock per SPMD call over an axon tunnel
(~100 MB/s H2D, ~30 MB/s D2H, ~12 ms per transferred array), so the design
minimizes shipped bytes AND array count per call:

  - ALL per-core inputs travel as ONE fp16 blob [N_ROWS, 512] (~2.8 MB/core),
    byte-punned with AP.bitcast on device:
      rows 0:1024     x fp16 [S, H]
      rows 1024:2048  emb8T: exp'd bias/mask, fp8e4 [S, S] ([k,q] layout)
      rows 2048:2816  weight shard, bf16 (1/8 of wqkvo|w1|w2, AllGathered
                      on-device over NeuronLink into a DRAM bounce buffer)
      rows 2816:2832  folded biases, f32
  - emb = exp(bias - rowmax)*mask*128 is precomputed on host: softmax is
    invariant to per-row scaling, so the rowmax shift + x128 centers the fp8
    dynamic range. Masked entries are exactly 0, so no -1e9 clamp or
    max-subtraction pass is needed on device.
  - v bias folded into the output-projection bias on host (sum p = 1), LN
    affine params folded into W/b as in the reference.
  - output fp16 (halves the donated zero buffer shipped in AND the result
    shipped back), staged in SBUF and written with a single DMA.

Per-core dataflow (S=1024, H=512, NH=8, DH=64, FFN=2048, P=128) is the
transposed-attention scheme: yT built with PE transposes; qT/kT = W.T@yT per
head-pair row-packed (K=64 x2) in the PE array; v_aug carries a ones column
so the PV matmul also produces softmax denominators; FFN keeps hT transposed
so no further transposes are needed. All matmul operands bf16 (full PE rate),
PSUM accumulation fp32.

The gathered weight DRAM layout is rank-interleaved (8 blocks of 768 rows:
wqkvo_s 256 | w1_s 256 | w2_s 256 each); SBUF loads un-interleave it with
strided DMA rearranges. use_collectives=False (CoreSim) ships the full
rank-interleaved weight region in the blob instead, so all downstream code
is identical.
"""

import os
import sys

for _p in ("/opt/trn_rl_repo", "/root/.axon_site/_ro/trn_rl_repo"):
    if os.path.isdir(_p) and _p not in sys.path:
        sys.path.insert(0, _p)

from contextlib import ExitStack

import numpy as np
import ml_dtypes

import concourse.bass as bass
import concourse.tile as tile
from concourse import bacc, mybir
from concourse.masks import make_identity

F32 = mybir.dt.float32
F16 = mybir.dt.float16
BF16 = mybir.dt.bfloat16
F8 = mybir.dt.float8e4
I8 = mybir.dt.int8
AF = mybir.ActivationFunctionType
ALU = mybir.AluOpType

S = 1024
H = 512
NH = 8
DH = 64
FFN = 2048
P = 128
B = 8
EPS = 1e-5
SSC = S // P     # 8 seq tiles of 128
CC = H // P      # 4 channel chunks
FT = FFN // P    # 16 ffn chunks
QC = S // 512    # 2 query chunks of 512

MM_DT = BF16     # matmul-operand dtype (full PE rate, fp32 PSUM accumulate)
EMB_SCALE = 128.0  # per-row softmax scale freedom used to center fp8 range

NP_BF16 = ml_dtypes.bfloat16
NP_F8 = ml_dtypes.float8_e4m3

# blob row offsets (f16 rows of 512 = 1KB each)
R_X = 0                     # x int8 [S, H] = 512 rows (bitcast)
R_EMB = 512
R_W = 1536
W_SHARD_ROWS = 768          # 256 wqkvo | 256 w1 | 256 w2 (bf16, as f16 rows)
W_FULL_ROWS = B * W_SHARD_ROWS


def _blob_rows(use_collectives):
    wrows = W_SHARD_ROWS if use_collectives else W_FULL_ROWS
    r_xs = R_W + wrows      # x per-row absmax f32 [S] = 4 rows
    r_ball = r_xs + 4
    r_bbc = r_ball + 12     # ball [128,24] f32 = 12 rows
    n = r_bbc + 4           # bbc [2,512] f32 = 4 rows
    return wrows, r_xs, r_ball, r_bbc, n


def build_program(use_collectives=True):
    nc = bacc.Bacc(
        "TRN2",
        target_bir_lowering=False,
        debug=False,
        enable_asserts=False,
        num_devices=B,
    )

    wrows, r_xs, r_ball, r_bbc, n_rows = _blob_rows(use_collectives)
    blob_d = nc.dram_tensor("blob", [n_rows, 512], F16, kind="ExternalInput").ap()
    # out rows 0:1024 int8 delta (out - x, per-seq-row absmax/127 quant),
    # rows 1024:1032 the f32 absmax values (bitcast), laid out [sc*128+p].
    out_d = nc.dram_tensor("out", [S + 8, H], I8, kind="ExternalOutput").ap()

    def _emit(tc, ctx):
        pool = ctx.enter_context(tc.tile_pool(name="main", bufs=1))
        stream = ctx.enter_context(tc.tile_pool(name="stream", bufs=2))
        spool = ctx.enter_context(tc.tile_pool(name="small", bufs=2))
        # PSUM: 2+2+2+2 slots = 8 banks exactly
        ps_mm = ctx.enter_context(tc.tile_pool(name="ps_mm", bufs=2, space="PSUM"))
        ps_s = ctx.enter_context(tc.tile_pool(name="ps_s", bufs=2, space="PSUM"))
        ps_o = ctx.enter_context(tc.tile_pool(name="ps_o", bufs=2, space="PSUM"))
        ps_sm = ctx.enter_context(tc.tile_pool(name="ps_sm", bufs=2, space="PSUM"))

        # ---- gather weight shards into one full rank-interleaved DRAM copy ----
        if use_collectives:
            dpool = ctx.enter_context(tc.tile_pool(name="dram", bufs=1, space="DRAM"))
            bin_t = dpool.tile([W_SHARD_ROWS, 512], MM_DT)
            g_t = dpool.tile([W_FULL_ROWS, 512], MM_DT)
            nc.gpsimd.dma_start(
                bin_t[:], blob_d[R_W:R_W + W_SHARD_ROWS].bitcast(MM_DT)
            )
            nc.gpsimd.collective_compute(
                "AllGather", ALU.bypass, replica_groups=[list(range(B))],
                ins=[bin_t.opt()], outs=[g_t.opt()],
            )
            wg = g_t[:]
        else:
            wg = blob_d[R_W:R_W + W_FULL_ROWS].bitcast(MM_DT)

        # ---- persistent SBUF tensors ----
        ident = pool.tile([P, P], F32, tag="ident")
        make_identity(nc, ident[:])
        x_sb = pool.tile([P, SSC, H], F32, tag="x")        # becomes x2 in place
        delta = pool.tile([P, SSC, H], F32, tag="delta")   # out - x (residual branches)
        scales_sb = pool.tile([P, SSC], F32, tag="scales")
        embT = pool.tile([P, SSC, S], F8, tag="big4mb")    # [k_in, kt, q], kept fp8
        yT = pool.tile([P, CC, S], MM_DT, tag="yT")          # [c_in, cc, s]
        v_aug = pool.tile([P, SSC, NH, DH + 1], MM_DT, tag="vaug")
        oT = pool.tile([P, CC, S], MM_DT, tag="oT")          # [c_in, cc, s]

        wq_sb = pool.tile([P, CC, H], MM_DT, tag="wslot0")
        wk_sb = pool.tile([P, CC, H], MM_DT, tag="wslot1")
        wv_sb = pool.tile([P, CC, H], MM_DT, tag="wslot2")
        wo_sb = pool.tile([P, CC, H], MM_DT, tag="wslot3")
        b_all = pool.tile([P, 24], F32, tag="ball")        # bq 0:4 | bk 4:8 | b1 8:24
        bo_row = pool.tile([1, H], F32, tag="bo_row")
        b2_row = pool.tile([1, H], F32, tag="b2_row")
        bo_sb = pool.tile([P, H], F32, tag="bo")
        b2_sb = pool.tile([P, H], F32, tag="b2")

        # wqkvo rows g live at rank g//256, inner g%256; [128,512] tiles are
        # 128-row aligned inside 256-row rank chunks: rank 2w+r' holds SBUF
        # chunks cc = 2r', 2r'+1 of weight w (DMA APs max 3 dims).
        for w_i, w_sb in enumerate((wq_sb, wk_sb, wv_sb, wo_sb)):
            for rr in range(2):
                blk = (2 * w_i + rr) * W_SHARD_ROWS
                src = wg[blk:blk + 256]                      # [256, 512]
                src = src.rearrange("(two p) h -> p two h", two=2)
                nc.sync.dma_start(w_sb[:, 2 * rr:2 * rr + 2], src)

        ball_src = blob_d[r_ball:r_ball + 12].bitcast(F32)   # [12, 256]
        ball_src = ball_src.rearrange("a b -> (a b)").rearrange("(p q) -> p q", q=24)
        nc.sync.dma_start(b_all[:], ball_src)
        bbc_src = blob_d[r_bbc:r_bbc + 4].bitcast(F32)       # [4, 256]
        bbc_src = bbc_src.rearrange("a b -> (a b)").rearrange("(p q) -> p q", q=H)
        nc.sync.dma_start(bo_row[:], bbc_src[0:1])
        nc.sync.dma_start(b2_row[:], bbc_src[1:2])
        nc.gpsimd.partition_broadcast(bo_sb[:], bo_row[:])
        nc.gpsimd.partition_broadcast(b2_sb[:], b2_row[:])

        # ---- load x (int8 + per-row absmax -> fp32) and emb (fp8 -> bf16) ----
        xq_t = pool.tile([P, SSC, H], I8, tag="x16")
        xq_src = blob_d[R_X:R_X + S // 2].bitcast(I8)        # [512, 1024]
        xq_src = xq_src.rearrange("a (two h) -> (a two) h", two=2)  # x rows [1024, 512]
        nc.sync.dma_start(
            xq_t[:], xq_src.rearrange("(sc p) h -> p sc h", p=P)
        )
        xsc_sb = pool.tile([P, SSC], F32, tag="xsc")
        xs_src = blob_d[r_xs:r_xs + 4].bitcast(F32)          # [4, 128]
        xs_src = xs_src.rearrange("a b -> (a b)").rearrange("(sc p) -> p sc", p=P)
        nc.sync.dma_start(xsc_sb[:], xs_src)
        nc.vector.tensor_scalar_mul(xsc_sb[:], xsc_sb[:], 1.0 / 127.0)
        for sc in range(SSC):
            nc.scalar.activation(
                x_sb[:, sc], xq_t[:, sc], AF.Identity, scale=xsc_sb[:, sc:sc + 1]
            )
        nc.sync.dma_start(
            embT[:],
            blob_d[R_EMB:R_EMB + S].bitcast(F8).rearrange("(kt p) q -> p kt q", p=P),
        )

        # ones columns of v_aug
        ones_col = pool.tile([P, 1], F32, tag="ones_col")
        nc.gpsimd.memset(ones_col[:], 1.0)
        nc.vector.tensor_copy(
            v_aug[:, :, :, DH:DH + 1],
            ones_col[:].to_broadcast((P, SSC, NH, 1)),
        )

        # ---- LN helper: batched stats for all SSC tiles in one pass ----
        def layer_norm_stats(src3):
            """src3: [P, SSC, H] fp32. Returns (rstd, nmr) [P, SSC] tiles with
            y = src*rstd + nmr the per-(partition, sc) normalization."""
            xsq = pool.tile([P, SSC, H], F32, tag="x16")
            nc.vector.tensor_tensor(xsq[:], src3, src3, ALU.mult)
            sumsq = spool.tile([P, SSC], F32, tag="sumsq")
            sumx = spool.tile([P, SSC], F32, tag="sumx")
            nc.vector.reduce_sum(sumsq[:], xsq[:], axis=mybir.AxisListType.X)
            nc.vector.reduce_sum(sumx[:], src3, axis=mybir.AxisListType.X)
            mean = spool.tile([P, SSC], F32, tag="mean")
            nc.vector.tensor_scalar_mul(mean[:], sumx[:], 1.0 / H)
            veps = spool.tile([P, SSC], F32, tag="veps")
            nc.vector.tensor_scalar_mul(veps[:], sumsq[:], 1.0 / H)
            msq = spool.tile([P, SSC], F32, tag="msq")
            nc.vector.tensor_tensor(msq[:], mean[:], mean[:], ALU.mult)
            nc.vector.tensor_tensor(veps[:], veps[:], msq[:], ALU.subtract)
            nc.vector.tensor_scalar_add(veps[:], veps[:], EPS)
            lnv = spool.tile([P, SSC], F32, tag="lnv")
            nc.scalar.activation(lnv[:], veps[:], AF.Ln)
            rstd = spool.tile([P, SSC], F32, tag="rstd")
            # rstd = exp(-0.5*ln(var+eps)); keeps ACT in the exp/ln table set
            nc.scalar.activation(rstd[:], lnv[:], AF.Exp, scale=-0.5)
            nmr = spool.tile([P, SSC], F32, tag="nmr")
            nc.vector.tensor_tensor(nmr[:], mean[:], rstd[:], ALU.mult)
            nc.vector.tensor_scalar_mul(nmr[:], nmr[:], -1.0)
            return rstd, nmr

        def transpose_into(y_tile, dst, sc):
            """PE-transpose y_tile [128, H] into dst [P, CC, S] at seq block sc.
            All CC transposes land in disjoint columns of one PSUM tile so a
            single strided copy evacuates them."""
            pst = ps_sm.tile([P, 512], F32, tag="ps_small")
            for cb in range(CC):
                nc.tensor.transpose(
                    pst[:, cb * P:(cb + 1) * P], y_tile[:, cb * P:(cb + 1) * P],
                    ident[:],
                )
            src = pst[:].rearrange("p (cb c) -> p cb c", cb=CC)
            if sc % 2 == 0:
                nc.scalar.copy(dst[:, :, sc * P:(sc + 1) * P], src)
            else:
                nc.vector.tensor_copy(dst[:, :, sc * P:(sc + 1) * P], src)

        # ---- LN1 + yT ----
        rstd1, nmr1 = layer_norm_stats(x_sb[:])
        for sc in range(SSC):
            y_t = stream.tile([P, H], F32, tag="y")
            nc.scalar.activation(
                y_t[:], x_sb[:, sc], AF.Identity,
                bias=nmr1[:, sc:sc + 1], scale=rstd1[:, sc:sc + 1],
            )
            transpose_into(y_t, yT, sc)

        # ---- V projection: v[s, h] = yT.T @ Wv, into v_aug slots ----
        for sc in range(SSC):
            psv = ps_mm.tile([P, H], F32, tag="mm")
            for ci in range(CC):
                nc.tensor.matmul(
                    psv[:],
                    yT[:, ci, sc * P:(sc + 1) * P],
                    wv_sb[:, ci],
                    start=(ci == 0),
                    stop=(ci == CC - 1),
                )
            # scatter per head into v_aug[:, sc, h, 0:64] (bv folded into bo')
            nc.vector.tensor_copy(
                v_aug[:, sc, :, 0:DH],
                psv[:].rearrange("p (h d) -> p h d", h=NH),
            )

        # ---- per head-pair: qT/kT projection then attention ----
        with tc.tile_pool(name="attnp", bufs=2) as attnp, \
             tc.tile_pool(name="epool", bufs=4) as epool:
            for cc in range(CC):
                h0, h1 = 2 * cc, 2 * cc + 1
                qT_c = attnp.tile([P, S], MM_DT, tag="qT")
                kT_c = attnp.tile([P, S], MM_DT, tag="kT")
                for qc in range(QC):
                    psq = ps_mm.tile([P, 512], F32, tag="mm")
                    for ci in range(CC):
                        nc.tensor.matmul(
                            psq[:],
                            wq_sb[:, ci, cc * P:(cc + 1) * P],
                            yT[:, ci, qc * 512:(qc + 1) * 512],
                            start=(ci == 0),
                            stop=(ci == CC - 1),
                        )
                    nc.scalar.activation(
                        qT_c[:, qc * 512:(qc + 1) * 512], psq[:], AF.Identity,
                        bias=b_all[:, cc:cc + 1],
                    )
                    psk = ps_mm.tile([P, 512], F32, tag="mm")
                    for ci in range(CC):
                        nc.tensor.matmul(
                            psk[:],
                            wk_sb[:, ci, cc * P:(cc + 1) * P],
                            yT[:, ci, qc * 512:(qc + 1) * 512],
                            start=(ci == 0),
                            stop=(ci == CC - 1),
                        )
                    nc.scalar.activation(
                        kT_c[:, qc * 512:(qc + 1) * 512], psk[:], AF.Identity,
                        bias=b_all[:, 4 + cc:5 + cc],
                    )

                for qc in range(QC):
                    qs = slice(qc * 512, (qc + 1) * 512)
                    # both heads' scores row-packed (K=64 at row groups 0-1 /
                    # 2-3) run concurrently in the PE array; their PV
                    # accumulation groups live in separate PSUM banks.
                    pso0 = ps_o.tile([DH + 1, 512], F32, tag="o", name=f"pso0_{qc}")
                    pso1 = ps_o.tile([DH + 1, 512], F32, tag="o", name=f"pso1_{qc}")
                    for kt in range(SSC):
                        ks = slice(kt * P, (kt + 1) * P)
                        pss0 = ps_s.tile([P, 512], F32, tag="s", name=f"pss0_{qc}_{kt}")
                        pss1 = ps_s.tile([P, 512], F32, tag="s", name=f"pss1_{qc}_{kt}")
                        nc.tensor.matmul(
                            pss0[:], kT_c[0:DH, ks], qT_c[0:DH, qs],
                            tile_position=(0, 0),
                        )
                        nc.tensor.matmul(
                            pss1[:], kT_c[DH:P, ks], qT_c[DH:P, qs],
                            tile_position=(DH, 0),
                        )
                        for h, pss, pso in ((h0, pss0, pso0), (h1, pss1, pso1)):
                            et = epool.tile([P, 512], MM_DT, tag="e", name=f"et_{h}_{qc}_{kt}")
                            nc.scalar.activation(et[:], pss[:], AF.Exp)
                            et2 = epool.tile([P, 512], MM_DT, tag="e2", name=f"et2_{h}_{qc}_{kt}")
                            nc.vector.tensor_tensor(
                                et2[:], et[:], embT[:, kt, qs], ALU.mult
                            )
                            nc.tensor.matmul(
                                pso[:],
                                v_aug[:, kt, h],
                                et2[:],
                                start=(kt == 0),
                                stop=(kt == SSC - 1),
                            )
                    for h, pso in ((h0, pso0), (h1, pso1)):
                        rows = slice(0, DH) if h == h0 else slice(DH, P)
                        r_t = spool.tile([1, 512], F32, tag="recip")
                        nc.vector.reciprocal(r_t[:], pso[DH:DH + 1, :])
                        bc_t = spool.tile([DH, 512], F32, tag="bc")
                        nc.gpsimd.partition_broadcast(bc_t[:], r_t[:])
                        nc.vector.tensor_tensor(
                            oT[rows, cc, qs], pso[0:DH, :], bc_t[:], ALU.mult
                        )

        # ---- output projection + residual (x2 overwrites x in place) ----
        for sc in range(SSC):
            pso = ps_mm.tile([P, H], F32, tag="mm")
            for ci in range(CC):
                nc.tensor.matmul(
                    pso[:],
                    oT[:, ci, sc * P:(sc + 1) * P],
                    wo_sb[:, ci],
                    start=(ci == 0),
                    stop=(ci == CC - 1),
                )
            nc.vector.tensor_tensor(delta[:, sc], pso[:], bo_sb[:], ALU.add)
            nc.gpsimd.tensor_tensor(x_sb[:, sc], x_sb[:, sc], delta[:, sc], ALU.add)

        # ---- LN2 + y2T (reuses the yT slot) ----
        y2T = pool.tile([P, CC, S], MM_DT, tag="yT")
        rstd2, nmr2 = layer_norm_stats(x_sb[:])
        for sc in range(SSC):
            y_t = stream.tile([P, H], F32, tag="y")
            nc.scalar.activation(
                y_t[:], x_sb[:, sc], AF.Identity,
                bias=nmr2[:, sc:sc + 1], scale=rstd2[:, sc:sc + 1],
            )
            transpose_into(y_t, y2T, sc)

        # ---- FFN ----
        # w1 [512, 2048]: rank r holds w1 rows r*64:(r+1)*64 as 256 blob rows
        # at rank-block offset 256. SBUF [p, cc, f]: p = (r%2)*64 + j, cc = r//2.
        w1_sb = pool.tile([P, CC, FFN], MM_DT, tag="w1")
        for r in range(B):
            src = wg[r * W_SHARD_ROWS + 256:r * W_SHARD_ROWS + 512]
            src = src.rearrange("(j four) h -> j (four h)", four=4)  # [64, 2048]
            nc.sync.dma_start(
                w1_sb[(r % 2) * 64:(r % 2) * 64 + 64, r // 2], src
            )
        # w2 [2048, 512]: rank r holds w2 rows r*256:(r+1)*256 at offset 512.
        w2_sb = pool.tile([P, FT, H], MM_DT, tag="vaug")
        for r in range(B):
            blk = r * W_SHARD_ROWS + 512
            src = wg[blk:blk + 256]                          # [256, 512]
            src = src.rearrange("(two p) h -> p two h", two=2)
            nc.sync.dma_start(w2_sb[:, 2 * r:2 * r + 2], src)
        hT = pool.tile([P, FT, S], MM_DT, tag="big4mb")

        for ft in range(FT):
            for qc in range(QC):
                psh = ps_mm.tile([P, 512], F32, tag="mm")
                for ci in range(CC):
                    nc.tensor.matmul(
                        psh[:],
                        w1_sb[:, ci, ft * P:(ft + 1) * P],
                        y2T[:, ci, qc * 512:(qc + 1) * 512],
                        start=(ci == 0),
                        stop=(ci == CC - 1),
                    )
                nc.scalar.activation(
                    hT[:, ft, qc * 512:(qc + 1) * 512], psh[:], AF.Gelu,
                    bias=b_all[:, 8 + ft:9 + ft],
                )

        o8 = pool.tile([P, SSC, H], I8, tag="x16")
        for sc in range(SSC):
            psf = ps_mm.tile([P, H], F32, tag="mm")
            for ft in range(FT):
                nc.tensor.matmul(
                    psf[:],
                    hT[:, ft, sc * P:(sc + 1) * P],
                    w2_sb[:, ft],
                    start=(ft == 0),
                    stop=(ft == FT - 1),
                )
            # full delta = attn branch + ffn branch; int8 per-row quantize
            d_t = stream.tile([P, H], F32, tag="out_t")
            nc.vector.tensor_tensor(d_t[:], psf[:], delta[:, sc], ALU.add)
            nc.gpsimd.tensor_tensor(d_t[:], d_t[:], b2_sb[:], ALU.add)
            nc.vector.tensor_reduce(
                scales_sb[:, sc:sc + 1], d_t[:], axis=mybir.AxisListType.X,
                op=ALU.max, apply_absolute_value=True,
            )
            r_t = spool.tile([P, 1], F32, tag="qr")
            nc.vector.reciprocal(r_t[:], scales_sb[:, sc:sc + 1])
            nc.vector.tensor_scalar_mul(r_t[:], r_t[:], 127.0)
            nc.scalar.activation(o8[:, sc], d_t[:], AF.Identity, scale=r_t[:])
        nc.sync.dma_start(out_d[0:S].rearrange("(sc p) h -> p sc h", p=P), o8[:])
        sc_dst = out_d[S:S + 8].bitcast(F32)                 # [8, 128]
        sc_dst = sc_dst.rearrange("a b -> (a b)").rearrange("(s p) -> p s", p=P)
        nc.sync.dma_start(sc_dst, scales_sb[:])

    with tile.TileContext(nc) as tc, ExitStack() as ctx:
        _emit(tc, ctx)

    nc.compile()
    return nc


def fold_weights(inputs):
    """Host-side prep: fold LN affine params, attention scale, and the V bias
    into weights/biases. Returns the rank-interleaved weight rows (as f16
    byte view) plus bias rows."""
    f = lambda a: np.asarray(a, np.float32)
    g1, b1l = f(inputs["ln1_g"]), f(inputs["ln1_b"])
    g2, b2l = f(inputs["ln2_g"]), f(inputs["ln2_b"])
    Wq, Wk, Wv, Wo = f(inputs["Wq"]), f(inputs["Wk"]), f(inputs["Wv"]), f(inputs["Wo"])
    W1, W2 = f(inputs["W1"]), f(inputs["W2"])
    scale = DH ** -0.5

    wq = (g1[:, None] * Wq) * scale
    bq = (b1l @ Wq + f(inputs["bq"])) * scale
    wk = g1[:, None] * Wk
    bk = b1l @ Wk + f(inputs["bk"])
    wv = g1[:, None] * Wv
    bv = b1l @ Wv + f(inputs["bv"])
    bo_eff = bv @ Wo + f(inputs["bo"])   # sum_k p_k = 1 folds bv through Wo
    w1 = g2[:, None] * W1
    b1_eff = b2l @ W1 + f(inputs["b1"])
    b2_eff = f(inputs["b2"])

    wqkvo = np.concatenate([wq, wk, wv, Wo], axis=0).astype(NP_BF16)  # [4H, H]
    w1c = w1.astype(NP_BF16)                                          # [H, FFN]
    w2c = W2.astype(NP_BF16)                                          # [FFN, H]
    # rank-interleaved blob rows (f16 byte view, 512 wide):
    # rank r block = [wqkvo rows r*256 | w1 rows r*64 (as 256 rows) | w2 rows r*256]
    shards = []
    for r in range(B):
        shards.append(np.concatenate([
            wqkvo[r * 256:(r + 1) * 256].view(np.float16),
            w1c[r * 64:(r + 1) * 64].view(np.float16).reshape(256, 512),
            w2c[r * 256:(r + 1) * 256].view(np.float16),
        ], axis=0))
    ball = np.zeros((P, 24), np.float32)
    ball[:, 0:4] = bq.reshape(CC, P).T
    ball[:, 4:8] = bk.reshape(CC, P).T
    ball[:, 8:24] = b1_eff.reshape(FT, P).T
    bias_rows = np.concatenate([
        ball.view(np.float16).reshape(12, 512),
        np.stack([bo_eff, b2_eff]).astype(np.float32).view(np.float16).reshape(4, 512),
    ], axis=0)
    return shards, bias_rows


def make_in_maps(inputs, use_collectives=True):
    """Build per-core input dicts: one fp16 blob per core."""
    shards, bias_rows = fold_weights(inputs)
    x = np.asarray(inputs["x"], np.float32)
    ab = np.asarray(inputs["attn_bias"], np.float32)
    gm = np.asarray(inputs["graph_mask"]) != 0            # [B, q, k]

    # emb[q,k] = exp(ab - rowmax_q) * mask * EMB_SCALE, in (0, EMB_SCALE].
    # Softmax over k is invariant to any per-q scale, so this only centers
    # the fp8 dynamic range (avoids both overflow and subnormal crush).
    masked = np.where(gm, ab, -np.inf)
    rowmax = masked.max(axis=-1, keepdims=True)           # [B, q, 1]
    rowmax = np.where(np.isfinite(rowmax), rowmax, 0.0)
    emb = np.exp(ab - rowmax) * gm * EMB_SCALE
    embT = np.swapaxes(emb, 1, 2)                         # [B, k, q]
    emb8 = np.ascontiguousarray(embT).astype(NP_F8)

    # x: per-seq-row absmax int8 quantization (reconstructed on device)
    x_absmax = np.abs(x).max(axis=-1)                     # [B, S]
    x_absmax = np.maximum(x_absmax, 1e-9)
    xq = np.clip(
        np.rint(x * (127.0 / x_absmax[..., None])), -127, 127
    ).astype(np.int8)                                     # [B, S, H]

    wrows, r_xs, r_ball, r_bbc, n_rows = _blob_rows(use_collectives)
    in_maps = []
    for b in range(B):
        blob = np.empty((n_rows, 512), np.float16)
        blob[R_X:R_X + S // 2] = xq[b].reshape(S // 2, 1024).view(np.float16)
        blob[R_EMB:R_EMB + S] = emb8[b].view(np.float16)
        if use_collectives:
            blob[R_W:R_W + W_SHARD_ROWS] = shards[b]
        else:
            blob[R_W:R_W + W_FULL_ROWS] = np.concatenate(shards, axis=0)
        blob[r_xs:r_xs + 4] = (
            x_absmax[b].astype(np.float32).view(np.float16).reshape(4, 512)
        )
        blob[r_ball:n_rows] = bias_rows
        in_maps.append({"blob": blob})
    return in_maps


_NC_CACHE = {}


def _get_nc(use_collectives=True):
    key = ("nc", use_collectives)
    if key not in _NC_CACHE:
        _NC_CACHE[key] = build_program(use_collectives)
    return _NC_CACHE[key]


def decode_out(raw, x_b):
    """raw: [S+8, 512] int8 device output -> fp32 [S, H] result.
    rows 0:S = int8 delta rows (seq-ordered), rows S:S+8 = f32 absmax
    per seq row (bitcast, seq-ordered). out = x + delta*absmax/127."""
    raw = np.asarray(raw)
    q = raw[0:S].astype(np.float32)
    absmax = np.frombuffer(raw[S:S + 8].tobytes(), np.float32)  # [S]
    return np.asarray(x_b, np.float32) + q * (absmax / 127.0)[:, None]


def kernel(**inputs) -> np.ndarray:
    from concourse import bass_utils

    nc = _get_nc()
    in_maps = make_in_maps(inputs)
    res = bass_utils.run_bass_kernel_spmd(nc, in_maps, core_ids=list(range(B)))
    x = np.asarray(inputs["x"], np.float32)
    return np.stack(
        [decode_out(res.results[b]["out"], x[b]) for b in range(B)], axis=0
    )


if __name__ == "__main__":
    nc = build_program()
    print("build+compile OK")
